# revision 5
# baseline (speedup 1.0000x reference)
"""Trainium2 Bass kernel for nn_Backbone_44994077393320 (topk_masking).

Pipeline (per the reference):
  start_reps = LN(gelu(X @ W_sm))          [S, FF]
  end_reps   = LN(gelu(X @ W_em))          [S, FF]
  temp       = start_reps @ W_s2e + b_s2e  [S, FF]
  joint      = temp @ end_reps.T           [S, S]   (only the s<=t<s+30 band matters)
  logits     = joint + start_logits[:,None] + end_logits[None,:]
  -> top-819 indices over the band, sigmoid/log losses over the band

Distribution: sequence-sharded over 8 cores (512 rows each + 29-row halo
for end_reps).  Each core computes its band panel [4, 128, 256] of logits;
the host extracts the diagonal band, does top-k + index sort, and the
(cheap) scalar loss, mirroring the reference's fp32 ops.

Precision: matmuls run as 3-pass hi/lo bf16 splits (hi=bf16(x),
lo=bf16(x-hi); passes hi*hi + lo*hi + hi*lo at fp32 PSUM accumulation).
This gives ~5.7e-4 band-logit noise vs a 1.2e-2 top-k boundary gap.
Single-pass fp32r (e8m11) would flip the top-k set.
"""

import numpy as np
import ml_dtypes
from contextlib import ExitStack

import concourse.bass as bass
import concourse.bacc as bacc
import concourse.tile as tile
from concourse import mybir
from concourse.bass_utils import run_bass_kernel_spmd
from concourse.masks import make_identity

F32 = mybir.dt.float32
BF16 = mybir.dt.bfloat16
AF = mybir.ActivationFunctionType
OP = mybir.AluOpType
BF = ml_dtypes.bfloat16

S, H, FF = 4096, 1024, 3072
SPAN = 30
NCORES = 8
ROWS = 512            # s rows per core
EROWS = ROWS + 32     # end rows incl. halo (29 needed, padded to 32)
ETC = 640             # padded end-T columns (4*128 + 256-128)
NKH = H // 128        # 8 k-tiles over H
NKF = FF // 128       # 24 k-tiles over FF
WCH = 256             # FCL weight-chunk width
NCW = FF // WCH       # 12 chunks
NBN = FF // 512       # bn_stats chunks
MAXK = 819
LN_EPS = 1e-5

# module-level cache of the built+compiled program
_PROG = {}
LAST_EXEC_NS = None
TRACE = False


def _split_bf16(x):
    hi = np.asarray(x, np.float32).astype(BF)
    lo = (np.asarray(x, np.float32) - hi.astype(np.float32)).astype(BF)
    return hi, lo


def _build(has_fcl_bias, has_gb):
    nc = bacc.Bacc("TRN2", target_bir_lowering=False, debug=False)

    x_d = nc.dram_tensor("x", [EROWS, H], F32, kind="ExternalInput").ap()
    wsm_h_d = nc.dram_tensor("wsm_h", [H, FF], BF16, kind="ExternalInput").ap()
    wsm_l_d = nc.dram_tensor("wsm_l", [H, FF], BF16, kind="ExternalInput").ap()
    wem_h_d = nc.dram_tensor("wem_h", [H, FF], BF16, kind="ExternalInput").ap()
    wem_l_d = nc.dram_tensor("wem_l", [H, FF], BF16, kind="ExternalInput").ap()
    w2_h_d = nc.dram_tensor("w2_h", [FF, FF], BF16, kind="ExternalInput").ap()
    w2_l_d = nc.dram_tensor("w2_l", [FF, FF], BF16, kind="ExternalInput").ap()
    wvs_h_d = nc.dram_tensor("wvs_h", [FF, 1], BF16, kind="ExternalInput").ap()
    wvs_l_d = nc.dram_tensor("wvs_l", [FF, 1], BF16, kind="ExternalInput").ap()
    # w_end + b_s2e (f32, added per-partition while evicting temp PSUM)
    wve_d = nc.dram_tensor("wve", [FF, 1], F32, kind="ExternalInput").ap()
    if has_gb:
        g_sm_d = nc.dram_tensor("g_sm", [FF], F32, kind="ExternalInput").ap()
        be_sm_d = nc.dram_tensor("be_sm", [FF], F32, kind="ExternalInput").ap()
        g_em_d = nc.dram_tensor("g_em", [FF], F32, kind="ExternalInput").ap()
        be_em_d = nc.dram_tensor("be_em", [FF], F32, kind="ExternalInput").ap()
    if has_fcl_bias:
        b_sm_d = nc.dram_tensor("b_sm", [FF], F32, kind="ExternalInput").ap()
        b_em_d = nc.dram_tensor("b_em", [FF], F32, kind="ExternalInput").ap()
    panels_d = nc.dram_tensor("panels", [4, 128, 256], F32, kind="ExternalOutput").ap()

    def bcast_rows(ap, p=128):
        # view a [N] dram AP as [p, N] with 0-stride partition dim
        return bass.AP(tensor=ap.tensor, offset=ap.offset, ap=[[0, p]] + list(ap.ap))

    def mm3(ps, lh, ll, rh, rl, first, last):
        nc.tensor.matmul(ps, lh, rh, start=first, stop=False)
        nc.tensor.matmul(ps, ll, rh, start=False, stop=False)
        nc.tensor.matmul(ps, lh, rl, start=False, stop=last)

    with tile.TileContext(nc) as tc, ExitStack() as ctx:
        const = ctx.enter_context(tc.tile_pool(name="const", bufs=1))
        ident = const.tile([128, 128], F32)
        make_identity(nc, ident[:])
        eps_t = const.tile([128, 1], F32)
        nc.vector.memset(eps_t[:], LN_EPS)

        # end-side transposed activations, persistent until the joint phase
        p_ret = ctx.enter_context(tc.tile_pool(name="p_ret", bufs=1))
        reT_h = [p_ret.tile([128, ETC], BF16, name=f"reTh{f}") for f in range(NKF)]
        reT_l = [p_ret.tile([128, ETC], BF16, name=f"reTl{f}") for f in range(NKF)]
        for f in range(NKF):
            nc.vector.memset(reT_h[f][:], 0.0)
            nc.vector.memset(reT_l[f][:], 0.0)

        fcl_ctx = ExitStack()
        p_xt = fcl_ctx.enter_context(tc.tile_pool(name="p_xt", bufs=1))
        p_wf = fcl_ctx.enter_context(tc.tile_pool(name="p_wf", bufs=1))
        p_ps_tr = fcl_ctx.enter_context(
            tc.tile_pool(name="p_ps_tr", bufs=4, space="PSUM"))
        p_ps_mm = fcl_ctx.enter_context(
            tc.tile_pool(name="p_ps_mm", bufs=2, space="PSUM"))

        # ---------------- phase A: X load + transpose + split ----------------
        xt_h = [p_xt.tile([128, EROWS], BF16, name=f"xth{k}") for k in range(NKH)]
        xt_l = [p_xt.tile([128, EROWS], BF16, name=f"xtl{k}") for k in range(NKH)]
        nrows = [128, 128, 128, 128, 32]
        with tc.tile_pool(name="p_xa", bufs=1) as p_xa:
            x_nat = []
            for i in range(5):
                xn = p_xa.tile([128, H], F32, name=f"xnat{i}")
                nc.sync.dma_start(out=xn[: nrows[i], :],
                                  in_=x_d[i * 128 : i * 128 + nrows[i], :])
                x_nat.append(xn)
            for k in range(NKH):
                for i in range(5):
                    r = nrows[i]
                    tp = p_ps_tr.tile([128, 128], F32, tag="trps", name=f"tp{k}_{i}")
                    nc.tensor.transpose(tp[:, :r], x_nat[i][:r, k * 128 : (k + 1) * 128],
                                        ident[:r, :r])
                    c0 = i * 128
                    nc.vector.tensor_copy(out=xt_h[k][:, c0 : c0 + r], in_=tp[:, :r])
                    nc.vector.scalar_tensor_tensor(
                        out=xt_l[k][:, c0 : c0 + r], in0=tp[:, :r], scalar=0.0,
                        in1=xt_h[k][:, c0 : c0 + r], op0=OP.add, op1=OP.subtract)

        # ---------------- phases B/C: the two FCLs (end side first) ----------
        rsT_h = rsT_l = None
        for side in ("e", "s"):
            nm = 5 if side == "e" else 4
            w_h_d, w_l_d = (wem_h_d, wem_l_d) if side == "e" else (wsm_h_d, wsm_l_d)
            with tc.tile_pool(name=f"p_side_{side}", bufs=1) as p_side:
                if has_gb:
                    g_d, be_d = (g_em_d, be_em_d) if side == "e" else (g_sm_d, be_sm_d)
                    g_rep = p_side.tile([128, FF], F32, name=f"grep_{side}")
                    be_rep = p_side.tile([128, FF], F32, name=f"berep_{side}")
                    nc.sync.dma_start(out=g_rep[:], in_=bcast_rows(g_d))
                    nc.sync.dma_start(out=be_rep[:], in_=bcast_rows(be_d))
                if has_fcl_bias:
                    b_d = b_em_d if side == "e" else b_sm_d
                    bias_rep = p_side.tile([128, FF], F32, name=f"biasrep_{side}")
                    nc.sync.dma_start(out=bias_rep[:], in_=bcast_rows(b_d))
                rnat = [p_side.tile([128, FF], F32, name=f"rnat{m}_{side}")
                        for m in range(nm)]
                w_h_r = w_h_d.rearrange("(k p) f -> p k f", p=128)
                w_l_r = w_l_d.rearrange("(k p) f -> p k f", p=128)
                for n in range(NCW):
                    wch = p_wf.tile([128, NKH, WCH], BF16, tag="wch", bufs=2,
                                    name=f"wch_{side}_{n}")
                    wcl = p_wf.tile([128, NKH, WCH], BF16, tag="wcl", bufs=2,
                                    name=f"wcl_{side}_{n}")
                    nc.sync.dma_start(out=wch[:], in_=w_h_r[:, :, n * WCH : (n + 1) * WCH])
                    nc.sync.dma_start(out=wcl[:], in_=w_l_r[:, :, n * WCH : (n + 1) * WCH])
                    for m in range(nm):
                        r = nrows[m] if side == "e" else 128
                        c0 = m * 128
                        ps = p_ps_mm.tile([128, WCH], F32, tag="fclps",
                                          name=f"ps_{side}_{n}_{m}")
                        for k in range(NKH):
                            mm3(ps[:r, :],
                                xt_h[k][:, c0 : c0 + r], xt_l[k][:, c0 : c0 + r],
                                wch[:, k, :], wcl[:, k, :],
                                first=(k == 0), last=(k == NKH - 1))
                        if has_fcl_bias:
                            nc.vector.tensor_add(
                                out=ps[:r, :], in0=ps[:r, :],
                                in1=bias_rep[:r, n * WCH : (n + 1) * WCH])
                        nc.scalar.activation(out=rnat[m][:r, n * WCH : (n + 1) * WCH],
                                             in_=ps[:r, :], func=AF.Gelu)
                # layernorm + transpose + split
                if side == "s":
                    rT_h, rT_l = rsT_h, rsT_l
                else:
                    rT_h, rT_l = reT_h, reT_l
                for m in range(nm):
                    r = nrows[m] if side == "e" else 128
                    stats = p_side.tile([128, NBN, 6], F32, tag="stats",
                                        name=f"st_{side}_{m}")
                    for i in range(NBN):
                        nc.vector.bn_stats(out=stats[:r, i, :],
                                           in_=rnat[m][:r, i * 512 : (i + 1) * 512])
                    mv = p_side.tile([128, 2], F32, tag="mv", name=f"mv_{side}_{m}")
                    nc.vector.bn_aggr(out=mv[:r, :], in_=stats[:r])
                    rstd = p_side.tile([128, 1], F32, tag="rstd", name=f"rstd_{side}_{m}")
                    nc.scalar.activation(out=rstd[:r], in_=mv[:r, 1:2], func=AF.Sqrt,
                                         bias=eps_t[:r])
                    nc.vector.reciprocal(out=rstd[:r], in_=rstd[:r])
                    if has_gb:
                        # (h - mu) * g, then * rstd + beta
                        nc.vector.scalar_tensor_tensor(
                            out=rnat[m][:r, :], in0=rnat[m][:r, :], scalar=mv[:r, 0:1],
                            in1=g_rep[:r, :], op0=OP.subtract, op1=OP.mult)
                        nc.vector.scalar_tensor_tensor(
                            out=rnat[m][:r, :], in0=rnat[m][:r, :], scalar=rstd[:r],
                            in1=be_rep[:r, :], op0=OP.mult, op1=OP.add)
                    else:
                        nc.vector.tensor_scalar(
                            out=rnat[m][:r, :], in0=rnat[m][:r, :],
                            scalar1=mv[:r, 0:1], scalar2=rstd[:r],
                            op0=OP.subtract, op1=OP.mult)
                    for f in range(NKF):
                        tp2 = p_ps_tr.tile([128, 128], F32, tag="trps",
                                           name=f"tp2_{side}_{m}_{f}")
                        nc.tensor.transpose(tp2[:, :r],
                                            rnat[m][:r, f * 128 : (f + 1) * 128],
                                            ident[:r, :r])
                        c0 = m * 128
                        nc.vector.tensor_copy(out=rT_h[f][:, c0 : c0 + r],
                                              in_=tp2[:, :r])
                        nc.vector.scalar_tensor_tensor(
                            out=rT_l[f][:, c0 : c0 + r], in0=tp2[:, :r], scalar=0.0,
                            in1=rT_h[f][:, c0 : c0 + r], op0=OP.add, op1=OP.subtract)
            if side == "e":
                # start-side transposed activations, persistent until temp phase;
                # created only now (after the end-side naturals were freed)
                p_rst = ctx.enter_context(
                    tc.tile_pool(name="p_rst", bufs=1, side="right"))
                rsT_h = [p_rst.tile([128, ROWS], BF16, name=f"rsTh{f}")
                         for f in range(NKF)]
                rsT_l = [p_rst.tile([128, ROWS], BF16, name=f"rsTl{f}")
                         for f in range(NKF)]

        fcl_ctx.close()

        # ---------------- phase D: start-logit row ----------------
        p_aug = ctx.enter_context(tc.tile_pool(name="p_aug", bufs=1))
        aug_sl = p_aug.tile([2, ROWS], BF16)        # rows: sl_hi, sl_lo
        aug_ones = p_aug.tile([2, ETC], BF16)
        nc.vector.memset(aug_ones[:], 1.0)
        with tc.tile_pool(name="p_lg", bufs=1) as p_lg, \
             tc.tile_pool(name="p_ps_lg", bufs=1, space="PSUM") as p_ps_lg:
            wvs_h = p_lg.tile([128, NKF, 1], BF16)
            wvs_l = p_lg.tile([128, NKF, 1], BF16)
            nc.sync.dma_start(out=wvs_h[:],
                              in_=wvs_h_d.rearrange("(k p) c -> p k c", p=128))
            nc.sync.dma_start(out=wvs_l[:],
                              in_=wvs_l_d.rearrange("(k p) c -> p k c", p=128))
            ps_s = p_ps_lg.tile([1, ROWS], F32)
            for k in range(NKF):
                mm3(ps_s[:], wvs_h[:, k, :], wvs_l[:, k, :],
                    rsT_h[k][:], rsT_l[k][:], first=(k == 0), last=(k == NKF - 1))
            sl_hi = p_lg.tile([1, ROWS], BF16)
            sl_lo = p_lg.tile([1, ROWS], BF16)
            nc.vector.tensor_copy(out=sl_hi[:], in_=ps_s[:])
            nc.vector.scalar_tensor_tensor(out=sl_lo[:], in0=ps_s[:], scalar=0.0,
                                           in1=sl_hi[:], op0=OP.add, op1=OP.subtract)
            nc.sync.dma_start(out=aug_sl[0:1, :], in_=sl_hi[:])
            nc.sync.dma_start(out=aug_sl[1:2, :], in_=sl_lo[:])

        # ------- phase E: temp.T = W_s2e.T @ rsT (+ wve added on eviction) -------
        p_tpt = ctx.enter_context(tc.tile_pool(name="p_tpt", bufs=1))
        tpT_h = [p_tpt.tile([128, ROWS], BF16, name=f"tpTh{o}") for o in range(NKF)]
        tpT_l = [p_tpt.tile([128, ROWS], BF16, name=f"tpTl{o}") for o in range(NKF)]
        with tc.tile_pool(name="p_w2", bufs=1) as p_w2, \
             tc.tile_pool(name="p_ps_t", bufs=2, space="PSUM") as p_ps_t:
            wve_sb = p_w2.tile([128, NKF, 1], F32)
            nc.sync.dma_start(out=wve_sb[:],
                              in_=wve_d.rearrange("(k p) c -> p k c", p=128))
            w2_h_r = w2_h_d.rearrange("(k p) o -> p k o", p=128)
            w2_l_r = w2_l_d.rearrange("(k p) o -> p k o", p=128)
            for o in range(NKF):
                w2ch = p_w2.tile([128, NKF, 128], BF16, tag="w2ch", bufs=2,
                                 name=f"w2ch{o}")
                w2cl = p_w2.tile([128, NKF, 128], BF16, tag="w2cl", bufs=2,
                                 name=f"w2cl{o}")
                nc.sync.dma_start(out=w2ch[:], in_=w2_h_r[:, :, o * 128 : (o + 1) * 128])
                nc.sync.dma_start(out=w2cl[:], in_=w2_l_r[:, :, o * 128 : (o + 1) * 128])
                ps_t = p_ps_t.tile([128, ROWS], F32, tag="tps", name=f"ps_t{o}")
                for k in range(NKF):
                    mm3(ps_t[:], w2ch[:, k, :], w2cl[:, k, :],
                        rsT_h[k][:], rsT_l[k][:], first=(k == 0), last=(k == NKF - 1))
                # hi = bf16(psum + wve); lo = bf16((psum + wve) - hi)
                nc.vector.tensor_scalar_add(out=tpT_h[o][:], in0=ps_t[:],
                                            scalar1=wve_sb[:, o, :])
                nc.vector.scalar_tensor_tensor(out=tpT_l[o][:], in0=ps_t[:],
                                               scalar=wve_sb[:, o, :],
                                               in1=tpT_h[o][:], op0=OP.add,
                                               op1=OP.subtract)

        # ---------------- phase F: joint band panels ----------------
        with tc.tile_pool(name="p_out", bufs=2) as p_out, \
             tc.tile_pool(name="p_ps_j", bufs=2, space="PSUM") as p_ps_j:
            for m in range(4):
                c0 = m * 128
                ps_j = p_ps_j.tile([128, 256], F32, tag="jps", name=f"ps_j{m}")
                for k in range(NKF):
                    mm3(ps_j[:],
                        tpT_h[k][:, c0 : c0 + 128], tpT_l[k][:, c0 : c0 + 128],
                        reT_h[k][:, c0 : c0 + 256], reT_l[k][:, c0 : c0 + 256],
                        first=(k == 0), last=False)
                nc.tensor.matmul(ps_j[:], aug_sl[:, c0 : c0 + 128],
                                 aug_ones[:, c0 : c0 + 256], start=False, stop=True)
                panel = p_out.tile([128, 256], F32, tag="panel", name=f"panel{m}")
                nc.scalar.copy(out=panel[:], in_=ps_j[:])
                nc.sync.dma_start(out=panels_d[m], in_=panel[:])

    nc.compile()
    return nc


def _get_program(has_fcl_bias, has_gb):
    key = ("prog", has_fcl_bias, has_gb)
    if key not in _PROG:
        _PROG[key] = _build(has_fcl_bias, has_gb)
    return _PROG[key]


def kernel(sequence_output, attention_mask, gold_mentions,
           W_sm, b_sm, g_sm, beta_sm, W_em, b_em, g_em, beta_em,
           w_start, b_start, w_end, b_end, W_s2e, b_s2e):
    global LAST_EXEC_NS
    sequence_output = np.asarray(sequence_output, np.float32)
    X = sequence_output[0]                                    # [S, H]
    gold = np.asarray(gold_mentions, np.int32)

    has_fcl_bias = bool(np.any(np.asarray(b_sm)) or np.any(np.asarray(b_em)))
    has_gb = not (np.all(np.asarray(g_sm) == 1) and np.all(np.asarray(beta_sm) == 0)
                  and np.all(np.asarray(g_em) == 1) and np.all(np.asarray(beta_em) == 0))
    nc = _get_program(has_fcl_bias, has_gb)

    wsm_h, wsm_l = _split_bf16(W_sm)
    wem_h, wem_l = _split_bf16(W_em)
    w2_h, w2_l = _split_bf16(W_s2e)
    wvs_h, wvs_l = _split_bf16(np.asarray(w_start, np.float32).reshape(FF, 1))
    wve = (np.asarray(w_end, np.float64) + np.asarray(b_s2e, np.float64)).astype(
        np.float32).reshape(FF, 1)

    shared = {
        "wsm_h": wsm_h, "wsm_l": wsm_l, "wem_h": wem_h, "wem_l": wem_l,
        "w2_h": w2_h, "w2_l": w2_l,
        "wvs_h": wvs_h, "wvs_l": wvs_l, "wve": wve,
    }
    if has_gb:
        shared["g_sm"] = np.asarray(g_sm, np.float32)
        shared["be_sm"] = np.asarray(beta_sm, np.float32)
        shared["g_em"] = np.asarray(g_em, np.float32)
        shared["be_em"] = np.asarray(beta_em, np.float32)
    if has_fcl_bias:
        shared["b_sm"] = np.asarray(b_sm, np.float32)
        shared["b_em"] = np.asarray(b_em, np.float32)

    xpad = np.zeros((S + 32, H), np.float32)
    xpad[:S] = X
    in_maps = []
    for c in range(NCORES):
        m = dict(shared)
        m["x"] = np.ascontiguousarray(xpad[c * ROWS : c * ROWS + EROWS])
        in_maps.append(m)

    res = run_bass_kernel_spmd(nc, in_maps, core_ids=list(range(NCORES)),
                               trace=TRACE)
    LAST_EXEC_NS = res.exec_time_ns
    panels = np.stack([res.results[c]["panels"] for c in range(NCORES)])  # [8,4,128,256]

    # ---- band extraction: band[s, d] = logits[s, s + d] ----
    P = panels.reshape(32, 128, 256)
    pidx = np.arange(128)[:, None]
    didx = np.arange(SPAN)[None, :]
    band = P[:, pidx, pidx + didx].reshape(S, SPAN).astype(np.float32)
    band += np.float32(np.asarray(b_start, np.float64) + np.asarray(b_end, np.float64))

    ss = np.arange(S)[:, None]
    valid = (ss + didx) < S
    vals = band[valid]
    flat = (ss * S + ss + didx)[valid]

    # ---- top-k (largest values, ties -> lowest flat index), then index sort ----
    order = np.lexsort((flat, -vals))[:MAXK]
    sel = np.sort(flat[order]).astype(np.int32)
    topk_start_ids = (sel // S).astype(np.int32)[None, :]
    topk_end_ids = (sel % S).astype(np.int32)[None, :]
    span_mask = np.ones((1, MAXK), np.float32)

    # ---- losses (fp32, mirroring the reference's jax ops) ----
    gs, ge = gold[:, 0], gold[:, 1]
    with np.errstate(over="ignore", divide="ignore"):
        probs = (np.float32(1.0)
                 / (np.float32(1.0) + np.exp(-band, dtype=np.float32))).astype(np.float32)
        gold_p = probs[gs, ge - gs]
        cost_gold = -np.maximum(np.log(gold_p, dtype=np.float32),
                                np.float32(-100.0)).mean(dtype=np.float32)
        junk = probs.copy()
        junk[gs, ge - gs] = np.float32(0.0)
        log1mp = np.maximum(
            np.log((np.float32(1.0) - junk).astype(np.float32), dtype=np.float32),
            np.float32(-100.0))
        log1mp = np.where(valid, log1mp, np.float32(0.0))
        cost_junk = -(log1mp.sum(dtype=np.float32) / np.float32(valid.sum()))
    cost_is_mention = np.asarray([np.float32(cost_gold) + np.float32(cost_junk)],
                                 np.float32)

    return (topk_start_ids, topk_end_ids, span_mask, sequence_output, cost_is_mention)


# revision 7
# speedup vs baseline: 1.0285x; 1.0285x over previous
"""Trainium2 Bass kernel for nn_Backbone_44994077393320 (topk_masking).

Pipeline (per the reference):
  start_reps = LN(gelu(X @ W_sm))          [S, FF]
  end_reps   = LN(gelu(X @ W_em))          [S, FF]
  temp       = start_reps @ W_s2e + b_s2e  [S, FF]
  joint      = temp @ end_reps.T           [S, S]   (only the s<=t<s+30 band matters)
  logits     = joint + start_logits[:,None] + end_logits[None,:]
  -> top-819 indices over the band, sigmoid/log losses over the band

Distribution: sequence-sharded over 8 cores (512 rows each + 29-row halo
for end_reps).  Each core computes its band panel [4, 128, 160] of logits;
the host extracts the diagonal band, does top-k + index sort, and the
(cheap) scalar loss, mirroring the reference's fp32 ops.

Precision: matmuls run as 3-pass hi/lo bf16 splits (hi=bf16(x),
lo=bf16(x-hi); passes hi*hi + lo*hi + hi*lo at fp32 PSUM accumulation).
This gives ~5.7e-4 band-logit noise vs a 1.2e-2 top-k boundary gap.
Single-pass fp32r (e8m11) would flip the top-k set.

end_logits and b_s2e are folded into the temp eviction (temp' = temp +
w_end + b_s2e, using sum_o end[t,o] independence... exactly:
J + el[t] = sum_o (temp[s,o] + w_end[o] + b_s2e[o]) * end[t,o]).
start_logits ride a K=2 augmented matmul ([sl_hi, sl_lo] x [ones, ones]).
Activations are transposed via 2-byte DMA-transpose (X-bar), not the PE.
"""

import numpy as np
import ml_dtypes
from contextlib import ExitStack

import concourse.bass as bass
import concourse.bacc as bacc
import concourse.tile as tile
from concourse import mybir
from concourse.bass_utils import run_bass_kernel_spmd
from concourse.masks import make_identity

F32 = mybir.dt.float32
BF16 = mybir.dt.bfloat16
AF = mybir.ActivationFunctionType
OP = mybir.AluOpType
BF = ml_dtypes.bfloat16

S, H, FF = 4096, 1024, 3072
SPAN = 30
NCORES = 8
ROWS = 512            # s rows per core
EROWS = ROWS + 32     # end rows incl. halo (29 needed, padded to 32)
NKH = H // 128        # 8 k-tiles over H
NKF = FF // 128       # 24 k-tiles over FF
WCH = 256             # FCL weight-chunk width
NCW = FF // WCH       # 12 chunks
JN = 160              # joint panel width (>= 128 + SPAN - 1 = 157)
MAXK = 819
LN_EPS = 1e-5

_PROG = {}
LAST_EXEC_NS = None
TRACE = False


def _split_bf16(x):
    hi = np.asarray(x, np.float32).astype(BF)
    lo = (np.asarray(x, np.float32) - hi.astype(np.float32)).astype(BF)
    return hi, lo


def _build(has_fcl_bias, has_gb):
    nc = bacc.Bacc("TRN2", target_bir_lowering=False, debug=False)

    x_d = nc.dram_tensor("x", [EROWS, H], F32, kind="ExternalInput").ap()
    wsm_h_d = nc.dram_tensor("wsm_h", [H, FF], BF16, kind="ExternalInput").ap()
    wsm_l_d = nc.dram_tensor("wsm_l", [H, FF], BF16, kind="ExternalInput").ap()
    wem_h_d = nc.dram_tensor("wem_h", [H, FF], BF16, kind="ExternalInput").ap()
    wem_l_d = nc.dram_tensor("wem_l", [H, FF], BF16, kind="ExternalInput").ap()
    w2_h_d = nc.dram_tensor("w2_h", [FF, FF], BF16, kind="ExternalInput").ap()
    w2_l_d = nc.dram_tensor("w2_l", [FF, FF], BF16, kind="ExternalInput").ap()
    wvs_h_d = nc.dram_tensor("wvs_h", [FF, 1], BF16, kind="ExternalInput").ap()
    wvs_l_d = nc.dram_tensor("wvs_l", [FF, 1], BF16, kind="ExternalInput").ap()
    # w_end + b_s2e (f32, added per-partition while evicting temp PSUM)
    wve_d = nc.dram_tensor("wve", [FF, 1], F32, kind="ExternalInput").ap()
    if has_gb:
        g_sm_d = nc.dram_tensor("g_sm", [FF], F32, kind="ExternalInput").ap()
        be_sm_d = nc.dram_tensor("be_sm", [FF], F32, kind="ExternalInput").ap()
        g_em_d = nc.dram_tensor("g_em", [FF], F32, kind="ExternalInput").ap()
        be_em_d = nc.dram_tensor("be_em", [FF], F32, kind="ExternalInput").ap()
    if has_fcl_bias:
        b_sm_d = nc.dram_tensor("b_sm", [FF], F32, kind="ExternalInput").ap()
        b_em_d = nc.dram_tensor("b_em", [FF], F32, kind="ExternalInput").ap()
    panels_d = nc.dram_tensor("panels", [4, 128, JN], F32, kind="ExternalOutput").ap()

    def bcast_rows(ap, p=128):
        # view a [N] dram AP as [p, N] with 0-stride partition dim
        return bass.AP(tensor=ap.tensor, offset=ap.offset, ap=[[0, p]] + list(ap.ap))

    def mm3(ps, lh, ll, rh, rl, first, last):
        nc.tensor.matmul(ps, lh, rh, start=first, stop=False)
        nc.tensor.matmul(ps, ll, rh, start=False, stop=False)
        nc.tensor.matmul(ps, lh, rl, start=False, stop=last)

    nrows = [128, 128, 128, 128, 32]

    with tile.TileContext(nc) as tc, ExitStack() as ctx:
        const = ctx.enter_context(tc.tile_pool(name="const", bufs=1))
        ident = const.tile([128, 128], F32)
        make_identity(nc, ident[:])
        eps_t = const.tile([128, 1], F32)
        nc.vector.memset(eps_t[:], LN_EPS)

        # transposed activations (bf16 hi/lo), [128, k, seq] layout
        p_ret = ctx.enter_context(tc.tile_pool(name="p_ret", bufs=1))
        reT_h = p_ret.tile([128, NKF, EROWS], BF16)
        reT_l = p_ret.tile([128, NKF, EROWS], BF16)

        p_aug = ctx.enter_context(tc.tile_pool(name="p_aug", bufs=1))
        aug_sl = p_aug.tile([2, ROWS], BF16)        # rows: sl_hi, sl_lo
        aug_ones = p_aug.tile([2, EROWS], BF16)
        nc.vector.memset(aug_ones[:], 1.0)

        fcl_ctx = ExitStack()
        p_xt = fcl_ctx.enter_context(tc.tile_pool(name="p_xt", bufs=1))
        p_ps_mm = fcl_ctx.enter_context(
            tc.tile_pool(name="p_ps_mm", bufs=2, space="PSUM"))

        # ---------------- phase A: X load + split + DMA-transpose -------------
        xt_h = p_xt.tile([128, NKH, EROWS], BF16)
        xt_l = p_xt.tile([128, NKH, EROWS], BF16)
        with tc.tile_pool(name="p_xa", bufs=1) as p_xa:
            for i in range(5):
                r = nrows[i]
                xn = p_xa.tile([128, H], F32, tag="xn", bufs=2, name=f"xnat{i}")
                nc.sync.dma_start(out=xn[:r, :], in_=x_d[i * 128 : i * 128 + r, :])
                xh = p_xa.tile([128, H], BF16, tag="xh", bufs=2, name=f"xh{i}")
                xl = p_xa.tile([128, H], BF16, tag="xl", bufs=2, name=f"xl{i}")
                nc.vector.tensor_copy(out=xh[:r, :], in_=xn[:r, :])
                nc.vector.scalar_tensor_tensor(out=xl[:r, :], in0=xn[:r, :],
                                               scalar=0.0, in1=xh[:r, :],
                                               op0=OP.add, op1=OP.subtract)
                c0 = i * 128
                nc.sync.dma_start_transpose(out=xt_h[:, :, c0 : c0 + r], in_=xh[:r, :])
                nc.sync.dma_start_transpose(out=xt_l[:, :, c0 : c0 + r], in_=xl[:r, :])

        # ---------------- phases B/C: the two FCLs (end side first) ----------
        rsT_h = rsT_l = None
        for side in ("e", "s"):
            nm = 5 if side == "e" else 4
            w_h_d, w_l_d = (wem_h_d, wem_l_d) if side == "e" else (wsm_h_d, wsm_l_d)
            with tc.tile_pool(name=f"p_side_{side}", bufs=1) as p_side, \
                 tc.tile_pool(name=f"p_wf_{side}", bufs=1) as p_wf:
                if has_gb:
                    g_d, be_d = (g_em_d, be_em_d) if side == "e" else (g_sm_d, be_sm_d)
                    g_rep = p_side.tile([128, FF], F32, name=f"grep_{side}")
                    be_rep = p_side.tile([128, FF], F32, name=f"berep_{side}")
                    nc.sync.dma_start(out=g_rep[:], in_=bcast_rows(g_d))
                    nc.sync.dma_start(out=be_rep[:], in_=bcast_rows(be_d))
                if has_fcl_bias:
                    b_d = b_em_d if side == "e" else b_sm_d
                    bias_rep = p_side.tile([128, FF], F32, name=f"biasrep_{side}")
                    nc.sync.dma_start(out=bias_rep[:], in_=bcast_rows(b_d))
                rnat = [p_side.tile([128, FF], F32, name=f"rnat{m}_{side}")
                        for m in range(nm)]
                stats = p_side.tile([128, nm, NCW, 6], F32)
                w_h_r = w_h_d.rearrange("(k p) f -> p k f", p=128)
                w_l_r = w_l_d.rearrange("(k p) f -> p k f", p=128)
                for n in range(NCW):
                    wch = p_wf.tile([128, NKH, WCH], BF16, tag="wch", bufs=2,
                                    name=f"wch_{side}_{n}")
                    wcl = p_wf.tile([128, NKH, WCH], BF16, tag="wcl", bufs=2,
                                    name=f"wcl_{side}_{n}")
                    nc.sync.dma_start(out=wch[:], in_=w_h_r[:, :, n * WCH : (n + 1) * WCH])
                    nc.sync.dma_start(out=wcl[:], in_=w_l_r[:, :, n * WCH : (n + 1) * WCH])
                    for m in range(nm):
                        r = nrows[m] if side == "e" else 128
                        c0 = m * 128
                        ps = p_ps_mm.tile([128, WCH], F32, tag="fclps",
                                          name=f"ps_{side}_{n}_{m}")
                        for k in range(NKH):
                            mm3(ps[:r, :],
                                xt_h[:, k, c0 : c0 + r], xt_l[:, k, c0 : c0 + r],
                                wch[:, k, :], wcl[:, k, :],
                                first=(k == 0), last=(k == NKH - 1))
                        if has_fcl_bias:
                            nc.vector.tensor_add(
                                out=ps[:r, :], in0=ps[:r, :],
                                in1=bias_rep[:r, n * WCH : (n + 1) * WCH])
                        nc.scalar.activation(out=rnat[m][:r, n * WCH : (n + 1) * WCH],
                                             in_=ps[:r, :], func=AF.Gelu)
                        nc.vector.bn_stats(out=stats[:r, m, n, :],
                                           in_=rnat[m][:r, n * WCH : (n + 1) * WCH])
                # ---- layernorm (batched rstd) + split + DMA-transpose ----
                mv = p_side.tile([128, nm, 2], F32)
                var_all = p_side.tile([128, nm], F32)
                for m in range(nm):
                    r = nrows[m] if side == "e" else 128
                    nc.vector.bn_aggr(out=mv[:r, m, :], in_=stats[:r, m])
                    nc.vector.tensor_copy(out=var_all[:r, m : m + 1],
                                          in_=mv[:r, m, 1:2])
                rstd_all = p_side.tile([128, nm], F32)
                nc.scalar.activation(out=rstd_all[:], in_=var_all[:], func=AF.Sqrt,
                                     bias=eps_t[:])
                nc.vector.reciprocal(out=rstd_all[:], in_=rstd_all[:])
                if side == "s":
                    rT_h, rT_l = rsT_h, rsT_l
                else:
                    rT_h, rT_l = reT_h, reT_l
                for m in range(nm):
                    r = nrows[m] if side == "e" else 128
                    c0 = m * 128
                    if has_gb:
                        nc.vector.scalar_tensor_tensor(
                            out=rnat[m][:r, :], in0=rnat[m][:r, :],
                            scalar=mv[:r, m, 0:1], in1=g_rep[:r, :],
                            op0=OP.subtract, op1=OP.mult)
                        nc.vector.scalar_tensor_tensor(
                            out=rnat[m][:r, :], in0=rnat[m][:r, :],
                            scalar=rstd_all[:r, m : m + 1], in1=be_rep[:r, :],
                            op0=OP.mult, op1=OP.add)
                    else:
                        nc.vector.tensor_scalar(
                            out=rnat[m][:r, :], in0=rnat[m][:r, :],
                            scalar1=mv[:r, m, 0:1], scalar2=rstd_all[:r, m : m + 1],
                            op0=OP.subtract, op1=OP.mult)
                    nh = p_side.tile([128, FF], BF16, tag="nh", bufs=2,
                                     name=f"nh_{side}_{m}")
                    nl = p_side.tile([128, FF], BF16, tag="nl", bufs=1,
                                     name=f"nl_{side}_{m}")
                    nc.scalar.copy(out=nh[:r, :], in_=rnat[m][:r, :])
                    nc.vector.scalar_tensor_tensor(
                        out=nl[:r, :], in0=rnat[m][:r, :], scalar=0.0,
                        in1=nh[:r, :], op0=OP.add, op1=OP.subtract)
                    nc.sync.dma_start_transpose(out=rT_h[:, :, c0 : c0 + r],
                                                in_=nh[:r, :])
                    nc.sync.dma_start_transpose(out=rT_l[:, :, c0 : c0 + r],
                                                in_=nl[:r, :])
            if side == "e":
                p_rst = ctx.enter_context(
                    tc.tile_pool(name="p_rst", bufs=1, side="right"))
                rsT_h = p_rst.tile([128, NKF, ROWS], BF16)
                rsT_l = p_rst.tile([128, NKF, ROWS], BF16)

        fcl_ctx.close()

        # ---------------- phase D: start-logit row -> aug rows ----------------
        with tc.tile_pool(name="p_lg", bufs=1) as p_lg, \
             tc.tile_pool(name="p_ps_lg", bufs=2, space="PSUM") as p_ps_lg:
            wvs_h = p_lg.tile([128, NKF, 1], BF16)
            wvs_l = p_lg.tile([128, NKF, 1], BF16)
            nc.sync.dma_start(out=wvs_h[:],
                              in_=wvs_h_d.rearrange("(k p) c -> p k c", p=128))
            nc.sync.dma_start(out=wvs_l[:],
                              in_=wvs_l_d.rearrange("(k p) c -> p k c", p=128))
            ps_s = p_ps_lg.tile([1, ROWS], F32)
            for k in range(NKF):
                mm3(ps_s[:], wvs_h[:, k, :], wvs_l[:, k, :],
                    rsT_h[:, k, :], rsT_l[:, k, :],
                    first=(k == 0), last=(k == NKF - 1))
            sl_hi = p_lg.tile([1, ROWS], BF16)
            sl_lo = p_lg.tile([1, ROWS], BF16)
            nc.vector.tensor_copy(out=sl_hi[:], in_=ps_s[:])
            nc.vector.scalar_tensor_tensor(out=sl_lo[:], in0=ps_s[:], scalar=0.0,
                                           in1=sl_hi[:], op0=OP.add, op1=OP.subtract)
            nc.sync.dma_start(out=aug_sl[0:1, :], in_=sl_hi[:])
            nc.sync.dma_start(out=aug_sl[1:2, :], in_=sl_lo[:])

        # ------- phase E: temp.T = W_s2e.T @ rsT (+ wve added on eviction) -------
        p_tpt = ctx.enter_context(tc.tile_pool(name="p_tpt", bufs=1))
        tpT_h = p_tpt.tile([128, NKF, ROWS], BF16)
        tpT_l = p_tpt.tile([128, NKF, ROWS], BF16)
        with tc.tile_pool(name="p_w2", bufs=1) as p_w2, \
             tc.tile_pool(name="p_ps_t", bufs=2, space="PSUM") as p_ps_t:
            wve_sb = p_w2.tile([128, NKF, 1], F32)
            nc.sync.dma_start(out=wve_sb[:],
                              in_=wve_d.rearrange("(k p) c -> p k c", p=128))
            w2_h_r = w2_h_d.rearrange("(k p) o -> p k o", p=128)
            w2_l_r = w2_l_d.rearrange("(k p) o -> p k o", p=128)
            for o in range(NKF):
                w2ch = p_w2.tile([128, NKF, 128], BF16, tag="w2ch", bufs=2,
                                 name=f"w2ch{o}")
                w2cl = p_w2.tile([128, NKF, 128], BF16, tag="w2cl", bufs=2,
                                 name=f"w2cl{o}")
                nc.sync.dma_start(out=w2ch[:], in_=w2_h_r[:, :, o * 128 : (o + 1) * 128])
                nc.sync.dma_start(out=w2cl[:], in_=w2_l_r[:, :, o * 128 : (o + 1) * 128])
                ps_t = p_ps_t.tile([128, ROWS], F32, tag="tps", name=f"ps_t{o}")
                for k in range(NKF):
                    mm3(ps_t[:], w2ch[:, k, :], w2cl[:, k, :],
                        rsT_h[:, k, :], rsT_l[:, k, :],
                        first=(k == 0), last=(k == NKF - 1))
                # hi = bf16(psum + wve); lo = bf16((psum + wve) - hi)
                nc.vector.tensor_scalar_add(out=tpT_h[:, o, :], in0=ps_t[:],
                                            scalar1=wve_sb[:, o, :])
                nc.vector.scalar_tensor_tensor(out=tpT_l[:, o, :], in0=ps_t[:],
                                               scalar=wve_sb[:, o, :],
                                               in1=tpT_h[:, o, :], op0=OP.add,
                                               op1=OP.subtract)

        # ---------------- phase F: joint band panels ----------------
        with tc.tile_pool(name="p_out", bufs=2) as p_out, \
             tc.tile_pool(name="p_ps_j", bufs=2, space="PSUM") as p_ps_j:
            for m in range(4):
                c0 = m * 128
                ps_j = p_ps_j.tile([128, JN], F32, tag="jps", name=f"ps_j{m}")
                for k in range(NKF):
                    mm3(ps_j[:],
                        tpT_h[:, k, c0 : c0 + 128], tpT_l[:, k, c0 : c0 + 128],
                        reT_h[:, k, c0 : c0 + JN], reT_l[:, k, c0 : c0 + JN],
                        first=(k == 0), last=False)
                nc.tensor.matmul(ps_j[:], aug_sl[:, c0 : c0 + 128],
                                 aug_ones[:, c0 : c0 + JN], start=False, stop=True)
                panel = p_out.tile([128, JN], F32, tag="panel", name=f"panel{m}")
                nc.scalar.copy(out=panel[:], in_=ps_j[:])
                nc.sync.dma_start(out=panels_d[m], in_=panel[:])

    nc.compile()
    return nc


def _get_program(has_fcl_bias, has_gb):
    key = ("prog", has_fcl_bias, has_gb)
    if key not in _PROG:
        _PROG[key] = _build(has_fcl_bias, has_gb)
    return _PROG[key]


def kernel(sequence_output, attention_mask, gold_mentions,
           W_sm, b_sm, g_sm, beta_sm, W_em, b_em, g_em, beta_em,
           w_start, b_start, w_end, b_end, W_s2e, b_s2e):
    global LAST_EXEC_NS
    sequence_output = np.asarray(sequence_output, np.float32)
    X = sequence_output[0]                                    # [S, H]
    gold = np.asarray(gold_mentions, np.int32)

    has_fcl_bias = bool(np.any(np.asarray(b_sm)) or np.any(np.asarray(b_em)))
    has_gb = not (np.all(np.asarray(g_sm) == 1) and np.all(np.asarray(beta_sm) == 0)
                  and np.all(np.asarray(g_em) == 1) and np.all(np.asarray(beta_em) == 0))
    nc = _get_program(has_fcl_bias, has_gb)

    wsm_h, wsm_l = _split_bf16(W_sm)
    wem_h, wem_l = _split_bf16(W_em)
    w2_h, w2_l = _split_bf16(W_s2e)
    wvs_h, wvs_l = _split_bf16(np.asarray(w_start, np.float32).reshape(FF, 1))
    wve = (np.asarray(w_end, np.float64) + np.asarray(b_s2e, np.float64)).astype(
        np.float32).reshape(FF, 1)

    shared = {
        "wsm_h": wsm_h, "wsm_l": wsm_l, "wem_h": wem_h, "wem_l": wem_l,
        "w2_h": w2_h, "w2_l": w2_l,
        "wvs_h": wvs_h, "wvs_l": wvs_l, "wve": wve,
    }
    if has_gb:
        shared["g_sm"] = np.asarray(g_sm, np.float32)
        shared["be_sm"] = np.asarray(beta_sm, np.float32)
        shared["g_em"] = np.asarray(g_em, np.float32)
        shared["be_em"] = np.asarray(beta_em, np.float32)
    if has_fcl_bias:
        shared["b_sm"] = np.asarray(b_sm, np.float32)
        shared["b_em"] = np.asarray(b_em, np.float32)

    xpad = np.zeros((S + 32, H), np.float32)
    xpad[:S] = X
    in_maps = []
    for c in range(NCORES):
        m = dict(shared)
        m["x"] = np.ascontiguousarray(xpad[c * ROWS : c * ROWS + EROWS])
        in_maps.append(m)

    res = run_bass_kernel_spmd(nc, in_maps, core_ids=list(range(NCORES)),
                               trace=TRACE)
    LAST_EXEC_NS = res.exec_time_ns
    panels = np.stack([res.results[c]["panels"] for c in range(NCORES)])  # [8,4,128,JN]

    # ---- band extraction: band[s, d] = logits[s, s + d] ----
    P = panels.reshape(32, 128, JN)
    pidx = np.arange(128)[:, None]
    didx = np.arange(SPAN)[None, :]
    band = P[:, pidx, pidx + didx].reshape(S, SPAN).astype(np.float32)
    band += np.float32(np.asarray(b_start, np.float64) + np.asarray(b_end, np.float64))

    ss = np.arange(S)[:, None]
    valid = (ss + didx) < S
    vals = band[valid]
    flat = (ss * S + ss + didx)[valid]

    # ---- top-k (largest values, ties -> lowest flat index), then index sort ----
    order = np.lexsort((flat, -vals))[:MAXK]
    sel = np.sort(flat[order]).astype(np.int32)
    topk_start_ids = (sel // S).astype(np.int32)[None, :]
    topk_end_ids = (sel % S).astype(np.int32)[None, :]
    span_mask = np.ones((1, MAXK), np.float32)

    # ---- losses (fp32, mirroring the reference's jax ops) ----
    gs, ge = gold[:, 0], gold[:, 1]
    with np.errstate(over="ignore", divide="ignore"):
        probs = (np.float32(1.0)
                 / (np.float32(1.0) + np.exp(-band, dtype=np.float32))).astype(np.float32)
        gold_p = probs[gs, ge - gs]
        cost_gold = -np.maximum(np.log(gold_p, dtype=np.float32),
                                np.float32(-100.0)).mean(dtype=np.float32)
        junk = probs.copy()
        junk[gs, ge - gs] = np.float32(0.0)
        log1mp = np.maximum(
            np.log((np.float32(1.0) - junk).astype(np.float32), dtype=np.float32),
            np.float32(-100.0))
        log1mp = np.where(valid, log1mp, np.float32(0.0))
        cost_junk = -(log1mp.sum(dtype=np.float32) / np.float32(valid.sum()))
    cost_is_mention = np.asarray([np.float32(cost_gold) + np.float32(cost_junk)],
                                 np.float32)

    return (topk_start_ids, topk_end_ids, span_mask, sequence_output, cost_is_mention)


# revision 9
# speedup vs baseline: 1.0490x; 1.0199x over previous
"""Trainium2 Bass kernel for nn_Backbone_44994077393320 (topk_masking).

Pipeline (per the reference):
  start_reps = LN(gelu(X @ W_sm))          [S, FF]
  end_reps   = LN(gelu(X @ W_em))          [S, FF]
  temp       = start_reps @ W_s2e + b_s2e  [S, FF]
  joint      = temp @ end_reps.T           [S, S]   (only the s<=t<s+30 band matters)
  logits     = joint + start_logits[:,None] + end_logits[None,:]
  -> top-819 indices over the band, sigmoid/log losses over the band

Distribution: sequence-sharded over 8 cores (512 rows each + 29-row halo
for end_reps).  Each core computes its band panel [4, 128, 160] of logits;
the host extracts the diagonal band, does top-k + index sort, and the
(cheap) scalar loss, mirroring the reference's fp32 ops.

Precision: matmuls run as 3-pass hi/lo bf16 splits (hi=bf16(x),
lo=bf16(x-hi); passes hi*hi + lo*hi + hi*lo at fp32 PSUM accumulation).
This gives ~5.7e-4 band-logit noise vs a 1.2e-2 top-k boundary gap.
Single-pass fp32r (e8m11) would flip the top-k set.

end_logits and b_s2e are folded into the temp eviction (temp' = temp +
w_end + b_s2e, using sum_o end[t,o] independence... exactly:
J + el[t] = sum_o (temp[s,o] + w_end[o] + b_s2e[o]) * end[t,o]).
start_logits ride a K=2 augmented matmul ([sl_hi, sl_lo] x [ones, ones]).
Activations are transposed via 2-byte DMA-transpose (X-bar), not the PE.
"""

import numpy as np
import ml_dtypes
from contextlib import ExitStack

import concourse.bass as bass
import concourse.bacc as bacc
import concourse.tile as tile
from concourse import mybir
from concourse.bass_utils import run_bass_kernel_spmd
from concourse.masks import make_identity

F32 = mybir.dt.float32
BF16 = mybir.dt.bfloat16
AF = mybir.ActivationFunctionType
OP = mybir.AluOpType
BF = ml_dtypes.bfloat16

S, H, FF = 4096, 1024, 3072
SPAN = 30
NCORES = 8
ROWS = 512            # s rows per core
EROWS = ROWS + 32     # end rows incl. halo (29 needed, padded to 32)
NKH = H // 128        # 8 k-tiles over H
NKF = FF // 128       # 24 k-tiles over FF
WCH = 256             # FCL weight-chunk width
NCW = FF // WCH       # 12 chunks
JN = 160              # joint panel width (>= 128 + SPAN - 1 = 157)
MAXK = 819
LN_EPS = 1e-5

_PROG = {}
LAST_EXEC_NS = None
TRACE = False


def _split_bf16(x):
    hi = np.asarray(x, np.float32).astype(BF)
    lo = (np.asarray(x, np.float32) - hi.astype(np.float32)).astype(BF)
    return hi, lo


def _build(has_fcl_bias, has_gb):
    nc = bacc.Bacc("TRN2", target_bir_lowering=False, debug=False)

    x_d = nc.dram_tensor("x", [EROWS, H], F32, kind="ExternalInput").ap()
    wsm_h_d = nc.dram_tensor("wsm_h", [H, FF], BF16, kind="ExternalInput").ap()
    wsm_l_d = nc.dram_tensor("wsm_l", [H, FF], BF16, kind="ExternalInput").ap()
    wem_h_d = nc.dram_tensor("wem_h", [H, FF], BF16, kind="ExternalInput").ap()
    wem_l_d = nc.dram_tensor("wem_l", [H, FF], BF16, kind="ExternalInput").ap()
    w2_h_d = nc.dram_tensor("w2_h", [FF, FF], BF16, kind="ExternalInput").ap()
    w2_l_d = nc.dram_tensor("w2_l", [FF, FF], BF16, kind="ExternalInput").ap()
    wvs_h_d = nc.dram_tensor("wvs_h", [FF, 1], BF16, kind="ExternalInput").ap()
    wvs_l_d = nc.dram_tensor("wvs_l", [FF, 1], BF16, kind="ExternalInput").ap()
    # w_end + b_s2e (f32, added per-partition while evicting temp PSUM)
    wve_d = nc.dram_tensor("wve", [FF, 1], F32, kind="ExternalInput").ap()
    if has_gb:
        g_sm_d = nc.dram_tensor("g_sm", [FF], F32, kind="ExternalInput").ap()
        be_sm_d = nc.dram_tensor("be_sm", [FF], F32, kind="ExternalInput").ap()
        g_em_d = nc.dram_tensor("g_em", [FF], F32, kind="ExternalInput").ap()
        be_em_d = nc.dram_tensor("be_em", [FF], F32, kind="ExternalInput").ap()
    if has_fcl_bias:
        b_sm_d = nc.dram_tensor("b_sm", [FF], F32, kind="ExternalInput").ap()
        b_em_d = nc.dram_tensor("b_em", [FF], F32, kind="ExternalInput").ap()
    panels_d = nc.dram_tensor("panels", [4, 128, JN], F32, kind="ExternalOutput").ap()
    scr_d = nc.dram_tensor("scr", [3, EROWS], F32, kind="Internal").ap()

    def bcast_rows(ap, p=128):
        # view a [N] dram AP as [p, N] with 0-stride partition dim
        return bass.AP(tensor=ap.tensor, offset=ap.offset, ap=[[0, p]] + list(ap.ap))

    def mm3(ps, lh, ll, rh, rl, first, last):
        nc.tensor.matmul(ps, lh, rh, start=first, stop=False)
        nc.tensor.matmul(ps, ll, rh, start=False, stop=False)
        nc.tensor.matmul(ps, lh, rl, start=False, stop=last)

    nrows = [128, 128, 128, 128, 32]

    with tile.TileContext(nc) as tc, ExitStack() as ctx:
        const = ctx.enter_context(tc.tile_pool(name="const", bufs=1))
        ident = const.tile([128, 128], F32)
        make_identity(nc, ident[:])
        eps_t = const.tile([128, 1], F32)
        nc.vector.memset(eps_t[:], LN_EPS)

        # transposed activations (bf16 hi/lo), [128, k, seq] layout
        p_ret = ctx.enter_context(tc.tile_pool(name="p_ret", bufs=1))
        reT_h = p_ret.tile([128, NKF, EROWS], BF16)
        reT_l = p_ret.tile([128, NKF, EROWS], BF16)
        rstd_s = p_ret.tile([128, 4], F32)
        rstd_e = p_ret.tile([128, 5], F32)


        fcl_ctx = ExitStack()
        p_xt = fcl_ctx.enter_context(tc.tile_pool(name="p_xt", bufs=1))
        p_wf = fcl_ctx.enter_context(tc.tile_pool(name="p_wf", bufs=1))
        p_ps_mm = fcl_ctx.enter_context(
            tc.tile_pool(name="p_ps_mm", bufs=2, space="PSUM"))

        # ---------------- phase A: X load + split + DMA-transpose -------------
        xt_h = p_xt.tile([128, NKH, EROWS], BF16)
        xt_l = p_xt.tile([128, NKH, EROWS], BF16)
        with tc.tile_pool(name="p_xa", bufs=1) as p_xa:
            for i in range(5):
                r = nrows[i]
                xn = p_xa.tile([128, H], F32, tag="xn", bufs=2, name=f"xnat{i}")
                nc.sync.dma_start(out=xn[:r, :], in_=x_d[i * 128 : i * 128 + r, :])
                xh = p_xa.tile([128, H], BF16, tag="xh", bufs=2, name=f"xh{i}")
                xl = p_xa.tile([128, H], BF16, tag="xl", bufs=2, name=f"xl{i}")
                nc.vector.tensor_copy(out=xh[:r, :], in_=xn[:r, :])
                nc.vector.scalar_tensor_tensor(out=xl[:r, :], in0=xn[:r, :],
                                               scalar=0.0, in1=xh[:r, :],
                                               op0=OP.add, op1=OP.subtract)
                c0 = i * 128
                nc.sync.dma_start_transpose(out=xt_h[:, :, c0 : c0 + r], in_=xh[:r, :])
                nc.sync.dma_start_transpose(out=xt_l[:, :, c0 : c0 + r], in_=xl[:r, :])

        # ---------------- phases B/C: the two FCLs (end side first) ----------
        rsT_h = rsT_l = None
        for side in ("e", "s"):
            nm = 5 if side == "e" else 4
            w_h_d, w_l_d = (wem_h_d, wem_l_d) if side == "e" else (wsm_h_d, wsm_l_d)
            with tc.tile_pool(name=f"p_side_{side}", bufs=1) as p_side:
                if has_gb:
                    g_d, be_d = (g_em_d, be_em_d) if side == "e" else (g_sm_d, be_sm_d)
                    g_rep = p_side.tile([128, FF], F32, name=f"grep_{side}")
                    be_rep = p_side.tile([128, FF], F32, name=f"berep_{side}")
                    nc.sync.dma_start(out=g_rep[:], in_=bcast_rows(g_d))
                    nc.sync.dma_start(out=be_rep[:], in_=bcast_rows(be_d))
                if has_fcl_bias:
                    b_d = b_em_d if side == "e" else b_sm_d
                    bias_rep = p_side.tile([128, FF], F32, name=f"biasrep_{side}")
                    nc.sync.dma_start(out=bias_rep[:], in_=bcast_rows(b_d))
                rnat = [p_side.tile([128, FF], F32, name=f"rnat{m}_{side}")
                        for m in range(nm)]
                stats = p_side.tile([128, nm, NCW, 6], F32)
                w_h_r = w_h_d.rearrange("(k p) f -> p k f", p=128)
                w_l_r = w_l_d.rearrange("(k p) f -> p k f", p=128)
                for n in range(NCW):
                    wch = p_wf.tile([128, NKH, WCH], BF16, tag="wch", bufs=2,
                                    name=f"wch_{side}_{n}")
                    wcl = p_wf.tile([128, NKH, WCH], BF16, tag="wcl", bufs=2,
                                    name=f"wcl_{side}_{n}")
                    nc.sync.dma_start(out=wch[:], in_=w_h_r[:, :, n * WCH : (n + 1) * WCH])
                    nc.sync.dma_start(out=wcl[:], in_=w_l_r[:, :, n * WCH : (n + 1) * WCH])
                    for m in range(nm):
                        r = nrows[m] if side == "e" else 128
                        c0 = m * 128
                        ps = p_ps_mm.tile([128, WCH], F32, tag="fclps",
                                          name=f"ps_{side}_{n}_{m}")
                        for k in range(NKH):
                            mm3(ps[:r, :],
                                xt_h[:, k, c0 : c0 + r], xt_l[:, k, c0 : c0 + r],
                                wch[:, k, :], wcl[:, k, :],
                                first=(k == 0), last=(k == NKH - 1))
                        if has_fcl_bias:
                            nc.vector.tensor_add(
                                out=ps[:r, :], in0=ps[:r, :],
                                in1=bias_rep[:r, n * WCH : (n + 1) * WCH])
                        nc.scalar.activation(out=rnat[m][:r, n * WCH : (n + 1) * WCH],
                                             in_=ps[:r, :], func=AF.Gelu)
                        nc.vector.bn_stats(out=stats[:r, m, n, :],
                                           in_=rnat[m][:r, n * WCH : (n + 1) * WCH])
                # ---- split z = (h - mu); rstd applied at PSUM evictions ----
                mv = p_side.tile([128, nm, 2], F32)
                negmu = p_side.tile([128, nm], F32)
                var_all = p_side.tile([128, nm], F32)
                nc.vector.memset(var_all[:], 1.0)
                for m in range(nm):
                    r = nrows[m] if side == "e" else 128
                    nc.vector.bn_aggr(out=mv[:r, m, :], in_=stats[:r, m])
                    nc.vector.tensor_scalar_mul(out=negmu[:r, m : m + 1],
                                                in0=mv[:r, m, 0:1], scalar1=-1.0)
                    nc.vector.tensor_copy(out=var_all[:r, m : m + 1],
                                          in_=mv[:r, m, 1:2])
                rstd_all = rstd_s if side == "s" else rstd_e
                sigma_all = p_side.tile([128, nm], F32)
                nc.scalar.activation(out=sigma_all[:], in_=var_all[:, :nm],
                                     func=AF.Sqrt, bias=eps_t[:])
                nc.vector.reciprocal(out=rstd_all[:], in_=sigma_all[:])
                if side == "s":
                    rT_h, rT_l = rsT_h, rsT_l
                else:
                    rT_h, rT_l = reT_h, reT_l
                for m in range(nm):
                    r = nrows[m] if side == "e" else 128
                    c0 = m * 128
                    nh = p_side.tile([128, FF], BF16, tag="nh", bufs=2,
                                     name=f"nh_{side}_{m}")
                    nl = p_side.tile([128, FF], BF16, tag="nl", bufs=2,
                                     name=f"nl_{side}_{m}")
                    if has_gb:
                        # z = (h - mu) * g + beta * sigma  (so z * rstd = LN out)
                        nc.vector.scalar_tensor_tensor(
                            out=rnat[m][:r, :], in0=rnat[m][:r, :],
                            scalar=mv[:r, m, 0:1], in1=g_rep[:r, :],
                            op0=OP.subtract, op1=OP.mult)
                        nc.vector.scalar_tensor_tensor(
                            out=rnat[m][:r, :], in0=be_rep[:r, :],
                            scalar=sigma_all[:r, m : m + 1], in1=rnat[m][:r, :],
                            op0=OP.mult, op1=OP.add)
                        nc.scalar.copy(out=nh[:r, :], in_=rnat[m][:r, :])
                        nc.vector.scalar_tensor_tensor(
                            out=nl[:r, :], in0=rnat[m][:r, :], scalar=0.0,
                            in1=nh[:r, :], op0=OP.add, op1=OP.subtract)
                    else:
                        # nh = bf16(h - mu) on ACT; nl = (h - mu) - nh on DVE
                        nc.scalar.activation(out=nh[:r, :], in_=rnat[m][:r, :],
                                             func=AF.Identity,
                                             bias=negmu[:r, m : m + 1])
                        nc.vector.scalar_tensor_tensor(
                            out=nl[:r, :], in0=rnat[m][:r, :],
                            scalar=mv[:r, m, 0:1],
                            in1=nh[:r, :], op0=OP.subtract, op1=OP.subtract)
                    nc.sync.dma_start_transpose(out=rT_h[:, :, c0 : c0 + r],
                                                in_=nh[:r, :])
                    nc.sync.dma_start_transpose(out=rT_l[:, :, c0 : c0 + r],
                                                in_=nl[:r, :])
            if side == "e":
                p_rst = ctx.enter_context(
                    tc.tile_pool(name="p_rst", bufs=1, side="right"))
                rsT_h = p_rst.tile([128, NKF, ROWS], BF16)
                rsT_l = p_rst.tile([128, NKF, ROWS], BF16)

        fcl_ctx.close()

        # -------- phase D: start-logit column sl[s] (scaled by rstd_s) --------
        p_sc = ctx.enter_context(tc.tile_pool(name="p_sc", bufs=1))
        sl_col = p_sc.tile([128, 4], F32)
        rstd_s_rep = p_sc.tile([128, ROWS], F32)
        rstd_e_rep = p_sc.tile([128, EROWS], F32)
        with tc.tile_pool(name="p_lg", bufs=1) as p_lg, \
             tc.tile_pool(name="p_ps_lg", bufs=2, space="PSUM") as p_ps_lg:
            wvs_h = p_lg.tile([128, NKF, 1], BF16)
            wvs_l = p_lg.tile([128, NKF, 1], BF16)
            nc.sync.dma_start(out=wvs_h[:],
                              in_=wvs_h_d.rearrange("(k p) c -> p k c", p=128))
            nc.sync.dma_start(out=wvs_l[:],
                              in_=wvs_l_d.rearrange("(k p) c -> p k c", p=128))
            ps_s = p_ps_lg.tile([1, ROWS], F32)
            for k in range(NKF):
                mm3(ps_s[:], wvs_h[:, k, :], wvs_l[:, k, :],
                    rsT_h[:, k, :], rsT_l[:, k, :],
                    first=(k == 0), last=(k == NKF - 1))
            # rstd cols [128, nm] -> DRAM rows (arbitrary strides on DRAM side)
            nc.sync.dma_start(
                out=bass.AP(tensor=scr_d.tensor, offset=scr_d.offset,
                            ap=[[1, 128], [128, 4]]),
                in_=rstd_s[:, 0:4])
            nc.sync.dma_start(
                out=bass.AP(tensor=scr_d.tensor, offset=scr_d.offset + EROWS,
                            ap=[[1, 128], [128, 4]]),
                in_=rstd_e[:, 0:4])
            nc.sync.dma_start(
                out=bass.AP(tensor=scr_d.tensor, offset=scr_d.offset + EROWS + 512,
                            ap=[[1, 32], [128, 1]]),
                in_=rstd_e[:32, 4:5])
            rstd_s_row = p_lg.tile([1, ROWS], F32)
            nc.sync.dma_start(out=rstd_s_row[:], in_=scr_d[0:1, 0:ROWS])
            slr = p_lg.tile([1, ROWS], F32)
            nc.vector.tensor_mul(out=slr[:], in0=ps_s[:], in1=rstd_s_row[:])
            # sl row -> DRAM -> per-partition column form [128, 4]
            nc.sync.dma_start(
                out=bass.AP(tensor=scr_d.tensor, offset=scr_d.offset + 2 * EROWS,
                            ap=[[1, ROWS]]),
                in_=slr[:])
            nc.sync.dma_start(
                out=sl_col[:],
                in_=bass.AP(tensor=scr_d.tensor, offset=scr_d.offset + 2 * EROWS,
                            ap=[[1, 128], [128, 4]]))
            # replicate rstd rows across partitions for the evictions
            nc.sync.dma_start(
                out=rstd_s_rep[:],
                in_=bass.AP(tensor=scr_d.tensor, offset=scr_d.offset,
                            ap=[[0, 128], [1, ROWS]]))
            nc.sync.dma_start(
                out=rstd_e_rep[:],
                in_=bass.AP(tensor=scr_d.tensor, offset=scr_d.offset + EROWS,
                            ap=[[0, 128], [1, EROWS]]))

        # ------- phase E: temp.T = W_s2e.T @ rsT (+ wve added on eviction) -------
        p_tpt = ctx.enter_context(tc.tile_pool(name="p_tpt", bufs=1))
        tpT_h = p_tpt.tile([128, NKF, ROWS], BF16)
        tpT_l = p_tpt.tile([128, NKF, ROWS], BF16)
        with tc.tile_pool(name="p_w2", bufs=1, side="right") as p_w2, \
             tc.tile_pool(name="p_ps_t", bufs=2, space="PSUM") as p_ps_t:
            wve_sb = p_w2.tile([128, NKF, 1], F32)
            nc.sync.dma_start(out=wve_sb[:],
                              in_=wve_d.rearrange("(k p) c -> p k c", p=128))
            w2_h_r = w2_h_d.rearrange("(k p) o -> p k o", p=128)
            w2_l_r = w2_l_d.rearrange("(k p) o -> p k o", p=128)
            for o in range(NKF):
                w2ch = p_w2.tile([128, NKF, 128], BF16, tag="w2ch", bufs=2,
                                 name=f"w2ch{o}")
                w2cl = p_w2.tile([128, NKF, 128], BF16, tag="w2cl", bufs=2,
                                 name=f"w2cl{o}")
                nc.sync.dma_start(out=w2ch[:], in_=w2_h_r[:, :, o * 128 : (o + 1) * 128])
                nc.sync.dma_start(out=w2cl[:], in_=w2_l_r[:, :, o * 128 : (o + 1) * 128])
                ps_t = p_ps_t.tile([128, ROWS], F32, tag="tps", name=f"ps_t{o}")
                for k in range(NKF):
                    mm3(ps_t[:], w2ch[:, k, :], w2cl[:, k, :],
                        rsT_h[:, k, :], rsT_l[:, k, :],
                        first=(k == 0), last=(k == NKF - 1))
                # u = psum * rstd_s (per-column scale), then +wve, split hi/lo
                nc.vector.tensor_mul(out=ps_t[:], in0=ps_t[:], in1=rstd_s_rep[:])
                nc.vector.tensor_scalar_add(out=tpT_h[:, o, :], in0=ps_t[:],
                                            scalar1=wve_sb[:, o, :])
                nc.vector.scalar_tensor_tensor(out=tpT_l[:, o, :], in0=ps_t[:],
                                               scalar=wve_sb[:, o, :],
                                               in1=tpT_h[:, o, :], op0=OP.add,
                                               op1=OP.subtract)

        # ---------------- phase F: joint band panels ----------------
        with tc.tile_pool(name="p_out", bufs=2) as p_out, \
             tc.tile_pool(name="p_ps_j", bufs=2, space="PSUM") as p_ps_j:
            for m in range(4):
                c0 = m * 128
                ps_j = p_ps_j.tile([128, JN], F32, tag="jps", name=f"ps_j{m}")
                for k in range(NKF):
                    mm3(ps_j[:],
                        tpT_h[:, k, c0 : c0 + 128], tpT_l[:, k, c0 : c0 + 128],
                        reT_h[:, k, c0 : c0 + JN], reT_l[:, k, c0 : c0 + JN],
                        first=(k == 0), last=(k == NKF - 1))
                panel = p_out.tile([128, JN], F32, tag="panel", name=f"panel{m}")
                nc.vector.tensor_mul(out=ps_j[:], in0=ps_j[:],
                                     in1=rstd_e_rep[:, c0 : c0 + JN])
                nc.vector.tensor_scalar_add(out=panel[:], in0=ps_j[:],
                                            scalar1=sl_col[:, m : m + 1])
                nc.sync.dma_start(out=panels_d[m], in_=panel[:])

    nc.compile()
    return nc


def _get_program(has_fcl_bias, has_gb):
    key = ("prog", has_fcl_bias, has_gb)
    if key not in _PROG:
        _PROG[key] = _build(has_fcl_bias, has_gb)
    return _PROG[key]


def kernel(sequence_output, attention_mask, gold_mentions,
           W_sm, b_sm, g_sm, beta_sm, W_em, b_em, g_em, beta_em,
           w_start, b_start, w_end, b_end, W_s2e, b_s2e):
    global LAST_EXEC_NS
    sequence_output = np.asarray(sequence_output, np.float32)
    X = sequence_output[0]                                    # [S, H]
    gold = np.asarray(gold_mentions, np.int32)

    has_fcl_bias = bool(np.any(np.asarray(b_sm)) or np.any(np.asarray(b_em)))
    has_gb = not (np.all(np.asarray(g_sm) == 1) and np.all(np.asarray(beta_sm) == 0)
                  and np.all(np.asarray(g_em) == 1) and np.all(np.asarray(beta_em) == 0))
    nc = _get_program(has_fcl_bias, has_gb)

    wsm_h, wsm_l = _split_bf16(W_sm)
    wem_h, wem_l = _split_bf16(W_em)
    w2_h, w2_l = _split_bf16(W_s2e)
    wvs_h, wvs_l = _split_bf16(np.asarray(w_start, np.float32).reshape(FF, 1))
    wve = (np.asarray(w_end, np.float64) + np.asarray(b_s2e, np.float64)).astype(
        np.float32).reshape(FF, 1)

    shared = {
        "wsm_h": wsm_h, "wsm_l": wsm_l, "wem_h": wem_h, "wem_l": wem_l,
        "w2_h": w2_h, "w2_l": w2_l,
        "wvs_h": wvs_h, "wvs_l": wvs_l, "wve": wve,
    }
    if has_gb:
        shared["g_sm"] = np.asarray(g_sm, np.float32)
        shared["be_sm"] = np.asarray(beta_sm, np.float32)
        shared["g_em"] = np.asarray(g_em, np.float32)
        shared["be_em"] = np.asarray(beta_em, np.float32)
    if has_fcl_bias:
        shared["b_sm"] = np.asarray(b_sm, np.float32)
        shared["b_em"] = np.asarray(b_em, np.float32)

    xpad = np.zeros((S + 32, H), np.float32)
    xpad[:S] = X
    in_maps = []
    for c in range(NCORES):
        m = dict(shared)
        m["x"] = np.ascontiguousarray(xpad[c * ROWS : c * ROWS + EROWS])
        in_maps.append(m)

    res = run_bass_kernel_spmd(nc, in_maps, core_ids=list(range(NCORES)),
                               trace=TRACE)
    LAST_EXEC_NS = res.exec_time_ns
    panels = np.stack([res.results[c]["panels"] for c in range(NCORES)])  # [8,4,128,JN]

    # ---- band extraction: band[s, d] = logits[s, s + d] ----
    P = panels.reshape(32, 128, JN)
    pidx = np.arange(128)[:, None]
    didx = np.arange(SPAN)[None, :]
    band = P[:, pidx, pidx + didx].reshape(S, SPAN).astype(np.float32)
    band += np.float32(np.asarray(b_start, np.float64) + np.asarray(b_end, np.float64))

    ss = np.arange(S)[:, None]
    valid = (ss + didx) < S
    vals = band[valid]
    flat = (ss * S + ss + didx)[valid]

    # ---- top-k (largest values, ties -> lowest flat index), then index sort ----
    order = np.lexsort((flat, -vals))[:MAXK]
    sel = np.sort(flat[order]).astype(np.int32)
    topk_start_ids = (sel // S).astype(np.int32)[None, :]
    topk_end_ids = (sel % S).astype(np.int32)[None, :]
    span_mask = np.ones((1, MAXK), np.float32)

    # ---- losses (fp32, mirroring the reference's jax ops) ----
    gs, ge = gold[:, 0], gold[:, 1]
    with np.errstate(over="ignore", divide="ignore"):
        probs = (np.float32(1.0)
                 / (np.float32(1.0) + np.exp(-band, dtype=np.float32))).astype(np.float32)
        gold_p = probs[gs, ge - gs]
        cost_gold = -np.maximum(np.log(gold_p, dtype=np.float32),
                                np.float32(-100.0)).mean(dtype=np.float32)
        junk = probs.copy()
        junk[gs, ge - gs] = np.float32(0.0)
        log1mp = np.maximum(
            np.log((np.float32(1.0) - junk).astype(np.float32), dtype=np.float32),
            np.float32(-100.0))
        log1mp = np.where(valid, log1mp, np.float32(0.0))
        cost_junk = -(log1mp.sum(dtype=np.float32) / np.float32(valid.sum()))
    cost_is_mention = np.asarray([np.float32(cost_gold) + np.float32(cost_junk)],
                                 np.float32)

    return (topk_start_ids, topk_end_ids, span_mask, sequence_output, cost_is_mention)
